# revision 1
# baseline (speedup 1.0000x reference)
"""GCN encoder (2-layer GCNConv + embedding lookup) on 8 trn2 NeuronCores.

Strategy (graph-parallel, per sharding hint):
  - Nodes are sharded across the 8 cores by id (12500 each), then renumbered
    into "slots": each core packs its nodes into G groups of <=128 nodes with
    balanced edge counts (bin-packing on host), giving a uniform SPMD program.
  - norm folding: out = dis .* segsum(u[src]) + b with u = dis .* (x @ W),
    dis = deg^-1/2 (self-loops included as ordinary edges).
  - u1 = dis * embW1[type] (embW1 = emb@W1 folded on host) computed per shard,
    AllGather -> full u1 table; per-group edge gathers (dma_gather, int16 idx
    with +-32768 midpoint base trick, 2 tables) feed one-hot matmul
    aggregation accumulated in PSUM; layer-2 the same with u2 = dis*(h1@W2).
"""
import os
import sys

sys.path.insert(0, "/opt/trn_rl_repo")
import numpy as np

N_NODES = 100000
NCORE = 8
NPC = N_NODES // NCORE          # 12500 nodes per core
D1, D2 = 128, 64
NTYPES = 1000
G = 104                         # groups per core
W = 128                         # slots (nodes) per group
SLOTS = G * W                   # 13312 slots per core
TOTAL_SLOTS = NCORE * SLOTS     # 106496
BASE_A, BASE_B = 32768, 73728   # gather base rows (midpoint trick)
SPLIT_NODE = 4 * NPC            # src node < 50000 -> table A (cores 0-3)
SPLIT_SLOT = 4 * G * 128        # first table-B slot
PAD_DST = 999.0
GB = 4                          # groups per dma_gather call                 # dst_local sentinel -> zero one-hot row


# ---------------------------------------------------------------- host prep
def _pack_core(nodes, degA, degB):
    """Greedy 2-d balanced packing of `nodes` into G groups of <=W nodes.
    Returns list of per-group node arrays."""
    a = degA[nodes].astype(np.float64)
    b = degB[nodes].astype(np.float64)
    order = np.argsort(-(a + b), kind="stable")
    tA = max(a.sum() / G, 1.0)
    tB = max(b.sum() / G, 1.0)
    sA = np.zeros(G)
    sB = np.zeros(G)
    cnt = np.zeros(G, np.int64)
    assign = np.empty(len(nodes), np.int64)
    for i in order:
        cost = np.maximum((sA + a[i]) / tA, (sB + b[i]) / tB)
        cost[cnt >= W] = np.inf
        g = int(np.argmin(cost))
        assign[i] = g
        sA[g] += a[i]
        sB[g] += b[i]
        cnt[g] += 1
    # refinement: push per-group section maxima down toward ceil(target/128)
    # by moving nodes out of the worst group into groups with slack.
    capA = max(1024, int(np.ceil(sA.max() / 128)) * 128 - 128)
    capB = max(1024, int(np.ceil(sB.max() / 128)) * 128 - 128)
    for _ in range(4000):
        worstA = sA.max() > capA
        s = sA if worstA else sB
        cap = capA if worstA else capB
        gsrc = int(np.argmax(s))
        if s[gsrc] <= cap:
            break
        members = np.where(assign == gsrc)[0]
        need = s[gsrc] - cap
        vals = (a if worstA else b)[members]
        cand = members[np.argsort(-(vals - need) * (vals >= need) - vals)]
        moved = False
        for i in cand[:20]:
            ai, bi = a[i], b[i]
            ok = (cnt < W) & (sA + ai <= capA) & (sB + bi <= capB)
            ok[gsrc] = False
            if ok.any():
                load = np.where(ok, np.maximum((sA + ai) / capA,
                                               (sB + bi) / capB), np.inf)
                gdst = int(np.argmin(load))
                assign[i] = gdst
                sA[gsrc] -= ai; sB[gsrc] -= bi; cnt[gsrc] -= 1
                sA[gdst] += ai; sB[gdst] += bi; cnt[gdst] += 1
                moved = True
                break
        if not moved:
            break
    groups = [nodes[assign == g] for g in range(G)]
    return groups


def preprocess(x_node_types, edge_index, emb, W1, b1, W2, b2):
    types = np.asarray(x_node_types).astype(np.int64)
    src = np.asarray(edge_index[0]).astype(np.int64)
    dst = np.asarray(edge_index[1]).astype(np.int64)
    loop = np.arange(N_NODES, dtype=np.int64)
    src_all = np.concatenate([src, loop])
    dst_all = np.concatenate([dst, loop])

    deg = np.bincount(dst_all, minlength=N_NODES).astype(np.float32)
    dis = (1.0 / np.sqrt(deg)).astype(np.float32)   # deg >= 1 (self loops)

    a_mask = src_all < SPLIT_NODE
    degA = np.bincount(dst_all[a_mask], minlength=N_NODES)
    degB = np.bincount(dst_all[~a_mask], minlength=N_NODES)

    # pack nodes -> slots
    slot_of = np.full(N_NODES, -1, np.int64)
    slot2node = np.full((NCORE, SLOTS), -1, np.int64)
    for c in range(NCORE):
        nodes = np.arange(c * NPC, (c + 1) * NPC, dtype=np.int64)
        groups = _pack_core(nodes, degA, degB)
        for g, gn in enumerate(groups):
            s0 = g * W
            slot_of[gn] = c * SLOTS + s0 + np.arange(len(gn))
            slot2node[c, s0:s0 + len(gn)] = gn

    # per-edge data
    e_srcslot = slot_of[src_all]
    e_dstslot = slot_of[dst_all]
    e_core = e_dstslot // SLOTS
    e_grp = (e_dstslot % SLOTS) // W
    e_dloc = e_dstslot % W
    e_sec = (src_all >= SPLIT_NODE).astype(np.int64)   # 0 = A, 1 = B

    bucket = (e_core * G + e_grp) * 2 + e_sec
    nb = NCORE * G * 2
    counts = np.bincount(bucket, minlength=nb)
    cA = counts.reshape(NCORE, G, 2)[:, :, 0]
    cB = counts.reshape(NCORE, G, 2)[:, :, 1]
    CA = int(np.ceil(cA.max() / 128))
    CB = int(np.ceil(cB.max() / 128))
    C = CA + CB

    # stable order by bucket; rank within bucket
    order = np.argsort(bucket, kind="stable")
    offs = np.zeros(nb + 1, np.int64)
    np.cumsum(counts, out=offs[1:])
    rank = np.arange(len(bucket)) - offs[bucket[order]]

    # padded per-(core,group,sec) slot position -> chunk & partition
    nsec = np.array([CA * 128, CB * 128])
    secbase = np.array([0, CA * 128])
    q = rank + secbase[e_sec[order]]          # slot index within group space
    part = q % 128
    chunk = q // 128

    # fill idx + dstl arrays (padded defaults)
    idx_val = np.zeros((NCORE, G, C * 128), np.int16)     # 0 = safe row at base
    dstl = np.full((NCORE, G, 128, C), PAD_DST, np.float32)
    oc = e_core[order]
    og = e_grp[order]
    osec = e_sec[order]
    oslot = e_srcslot[order]
    base = np.where(osec == 0, BASE_A, BASE_B)
    iv = oslot - base
    assert iv.min() >= -32768 and iv.max() <= 32767
    idx_val[oc, og, q] = iv.astype(np.int16)
    dstl[oc, og, part, chunk] = e_dloc[order].astype(np.float32)
    # guarantee the LAST index of each section is >= 0 (gen truncates
    # trailing negatives): pad slots already 0; if a real edge with negative
    # idx lands exactly at the section end, swap it with a pad... instead just
    # check and fix by appending nothing: force last element of each section
    # to be a pad when the section is full AND negative.
    lastA = idx_val[:, :, CA * 128 - 1]
    lastB = idx_val[:, :, C * 128 - 1]
    if (lastA < 0).any() or (lastB < 0).any():
        # swap the offending last element with the first non-negative element
        for c, g in zip(*np.where(lastA < 0)):
            sec = idx_val[c, g, :CA * 128]
            j = int(np.argmax(sec >= 0))
            _swap_edge(idx_val, dstl, c, g, CA * 128 - 1, j)
        for c, g in zip(*np.where(lastB < 0)):
            sec = idx_val[c, g, CA * 128:]
            j = CA * 128 + int(np.argmax(sec >= 0))
            _swap_edge(idx_val, dstl, c, g, C * 128 - 1, j)

    # wrapped int16 layout [128, n/16] (16-partition blocks replicated 8x)
    def wrap(vals):  # vals [..., n] -> [..., 128, n//16]
        n = vals.shape[-1]
        w = vals.reshape(*vals.shape[:-1], n // 16, 16)
        w = np.swapaxes(w, -1, -2)              # [..., 16, n//16]
        return np.tile(w, (1,) * (vals.ndim - 1) + (8, 1))

    # batched gather inputs: GB groups per dma_gather call, A and B sections
    # concatenated separately (wrap is 16-blockwise, so wrap(concat)=concat(wrap))
    idxA = idx_val[:, :, :CA * 128].reshape(NCORE, G // GB, GB * CA * 128)
    idxB = idx_val[:, :, CA * 128:].reshape(NCORE, G // GB, GB * CB * 128)
    gidxA = np.ascontiguousarray(wrap(idxA))   # [NCORE, G/GB, 128, GB*CA*8]
    gidxB = np.ascontiguousarray(wrap(idxB))   # [NCORE, G/GB, 128, GB*CB*8]
    gidx = np.ascontiguousarray(wrap(idx_val).transpose(0, 2, 1, 3).reshape(
        NCORE, 128, G * C * 8))                # [NCORE, 128, G*C*8]
    dstl_pc = np.ascontiguousarray(
        dstl.transpose(0, 2, 1, 3).reshape(NCORE, 128, G * C))

    # u1-phase gather: idx-list position i writes tile [p=i%128, c=i//128],
    # and the store DMA maps tile [p, c] -> u1_stage row p*NU+c.
    NU = SLOTS // 128
    ii = np.arange(SLOTS)
    slot_at_i = (ii % 128) * NU + ii // 128
    types_slot = np.zeros((NCORE, SLOTS), np.int64)
    dis_slot = np.zeros((NCORE, SLOTS), np.float32)
    for c in range(NCORE):
        valid = slot2node[c] >= 0
        types_slot[c, valid] = types[slot2node[c, valid]]
        dis_slot[c, valid] = dis[slot2node[c, valid]]
    ut_idx = wrap(types_slot[:, slot_at_i].astype(np.int16))  # [NCORE, 128, SLOTS//16]
    # dis_pc[c][p][j] = dis of slot p*NU+j
    dis_pc = dis_slot[:, (np.arange(128)[:, None] * NU + np.arange(NU)[None, :])]

    disb = np.ascontiguousarray(
        np.broadcast_to(
            dis_slot.reshape(NCORE, G, 1, W), (NCORE, G, 128, W)
        )
    ).astype(np.float32)
    dis_cols = np.ascontiguousarray(
        dis_slot.reshape(NCORE, G, W).transpose(0, 2, 1)
    ).astype(np.float32)                                   # [NCORE, 128, G]

    embW1 = (np.asarray(emb, np.float32) @ np.asarray(W1, np.float32)).astype(np.float32)
    iota_bc = np.tile(np.arange(W, dtype=np.float32)[None, :], (128, 1))
    b1c = np.asarray(b1, np.float32).reshape(128, 1)
    b2r = np.tile(np.asarray(b2, np.float32)[None, :], (128, 1))

    return dict(
        CA=CA, CB=CB, dis=dis, slot2node=slot2node,
        gidxA=gidxA, gidxB=gidxB, gidx=gidx, dstl_pc=dstl_pc,
        dstl=np.ascontiguousarray(dstl.reshape(NCORE, G, 128, C)),
        ut_idx=np.ascontiguousarray(ut_idx),
        dis_pc=np.ascontiguousarray(dis_pc.astype(np.float32)),
        disb=disb, dis_cols=dis_cols,
        embW1=embW1, iota_bc=iota_bc, b1c=b1c, b2r=b2r,
        w2=np.asarray(W2, np.float32),
    )


def _swap_edge(idx_val, dstl, c, g, i, j):
    C = dstl.shape[-1]
    idx_val[c, g, i], idx_val[c, g, j] = idx_val[c, g, j], idx_val[c, g, i]
    pi, ci, pj, cj = i % 128, i // 128, j % 128, j // 128
    t = dstl[c, g, pi, ci]
    dstl[c, g, pi, ci] = dstl[c, g, pj, cj]
    dstl[c, g, pj, cj] = t


# ---------------------------------------------------------------- device
def build_program(CA, CB):
    from concourse import bacc, mybir, tile

    C = CA + CB
    f32, i16 = mybir.dt.float32, mybir.dt.int16
    NU = SLOTS // 128
    NBATCH = G // GB

    nc = bacc.Bacc(None, target_bir_lowering=False, num_devices=NCORE,
                   num_swdge_queues=4)
    embw1_in = nc.dram_tensor("embw1", [NTYPES, D1], f32, kind="ExternalInput")
    w2_in = nc.dram_tensor("w2", [D1, D2], f32, kind="ExternalInput")
    gidx_in = nc.dram_tensor("gidx", [128, G * (CA + CB) * 8], i16,
                             kind="ExternalInput")
    dstlpc_in = nc.dram_tensor("dstlpc", [128, G * (CA + CB)], f32,
                               kind="ExternalInput")
    utidx_in = nc.dram_tensor("utidx", [128, SLOTS // 16], i16, kind="ExternalInput")
    dispc_in = nc.dram_tensor("dispc", [128, NU], f32, kind="ExternalInput")
    disb_in = nc.dram_tensor("disb", [G, 128, W], f32, kind="ExternalInput")
    discols_in = nc.dram_tensor("discols", [128, G], f32, kind="ExternalInput")
    iota_in = nc.dram_tensor("iota", [128, W], f32, kind="ExternalInput")
    b1c_in = nc.dram_tensor("b1c", [128, 1], f32, kind="ExternalInput")
    b2r_in = nc.dram_tensor("b2r", [128, D2], f32, kind="ExternalInput")
    out_ext = nc.dram_tensor("out", [SLOTS, D2], f32, kind="ExternalOutput")

    u1_stage = nc.dram_tensor("u1_stage", [SLOTS, D1], f32)
    u1_full = nc.dram_tensor("u1_full", [TOTAL_SLOTS, D1], f32, addr_space="Shared")
    u2_stage = nc.dram_tensor("u2_stage", [SLOTS, D2], f32)
    u2_full = nc.dram_tensor("u2_full", [TOTAL_SLOTS, D2], f32, addr_space="Shared")

    RG = [list(range(NCORE))]
    Relu = mybir.ActivationFunctionType.Relu
    Copy = mybir.ActivationFunctionType.Copy

    def sec_gather(gat, gq, g, table, elem, base_q):
        """Two per-group section gathers; idx slices of the resident gidx tile."""
        o = g * (CA + CB) * 8
        nc.gpsimd.dma_gather(
            out_ap=gat[:, :CA * elem].rearrange("p (c d) -> p c d", d=elem),
            in_ap=table[BASE_A:, :],
            idxs_ap=gq[:, o:o + CA * 8],
            num_idxs=CA * 128, num_idxs_reg=CA * 128,
            elem_size=elem, single_packet=False, queue_num=base_q % 4,
        )
        nc.gpsimd.dma_gather(
            out_ap=gat[:, CA * elem:].rearrange("p (c d) -> p c d", d=elem),
            in_ap=table[BASE_B:, :],
            idxs_ap=gq[:, o + CA * 8:o + (CA + CB) * 8],
            num_idxs=CB * 128, num_idxs_reg=CB * 128,
            elem_size=elem, single_packet=False, queue_num=(base_q + 1) % 4,
        )

    C_ = C

    ccw_in = nc.dram_tensor("ccw_in", [1, 128], f32)
    ccw_out = nc.dram_tensor("ccw_out", [NCORE, 128], f32, addr_space="Shared")

    with tile.TileContext(nc) as tc:
        with tc.tile_pool(name="cst", bufs=1) as cst:
            nc.gpsimd.collective_compute(
                "AllGather", mybir.AluOpType.bypass, replica_groups=RG,
                ins=[ccw_in[:]], outs=[ccw_out[:]],
            )
            w2_t = cst.tile([D1, D2], f32)
            nc.sync.dma_start(out=w2_t[:], in_=w2_in[:])
            iota_t = cst.tile([128, W], f32)
            nc.sync.dma_start(out=iota_t[:], in_=iota_in[:])
            b1c_t = cst.tile([128, 1], f32)
            nc.sync.dma_start(out=b1c_t[:], in_=b1c_in[:])
            b2r_t = cst.tile([128, D2], f32)
            nc.sync.dma_start(out=b2r_t[:], in_=b2r_in[:])
            discols_t = cst.tile([128, G], f32)
            nc.sync.dma_start(out=discols_t[:], in_=discols_in[:])
            gidx_t = cst.tile([128, G * C * 8], i16)
            nc.sync.dma_start(out=gidx_t[:], in_=gidx_in[:])
            dstl_t = cst.tile([128, G * C], f32)
            nc.sync.dma_start(out=dstl_t[:], in_=dstlpc_in[:])

            # ---- u1 = dis * embW1[type] (own pool, freed before main loops)
            with tc.tile_pool(name="u1p", bufs=1) as u1p:
                dispc_t = u1p.tile([128, NU], f32)
                nc.sync.dma_start(out=dispc_t[:], in_=dispc_in[:])
                utidx_t = u1p.tile([128, SLOTS // 16], i16)
                nc.sync.dma_start(out=utidx_t[:], in_=utidx_in[:])
                u1t = u1p.tile([128, NU * D1], f32)
                nc.gpsimd.dma_gather(
                    out_ap=u1t[:].rearrange("p (c d) -> p c d", d=D1),
                    in_ap=embw1_in[:],
                    idxs_ap=utidx_t[:],
                    num_idxs=SLOTS, num_idxs_reg=SLOTS,
                    elem_size=D1, single_packet=False, queue_num=0,
                )
                nc.vector.tensor_tensor(
                    out=u1t[:].rearrange("p (c d) -> p c d", d=D1),
                    in0=u1t[:].rearrange("p (c d) -> p c d", d=D1),
                    in1=dispc_t[:][:, :, None].to_broadcast([128, NU, D1]),
                    op=mybir.AluOpType.mult,
                )
                nc.sync.dma_start(
                    out=u1_stage[:].rearrange("(p c) d -> p c d", p=128),
                    in_=u1t[:].rearrange("p (c d) -> p c d", d=D1),
                )
            nc.gpsimd.collective_compute(
                "AllGather", mybir.AluOpType.bypass, replica_groups=RG,
                ins=[u1_stage[:]], outs=[u1_full[:]],
            )

            with tc.tile_pool(name="gat", bufs=8) as gatp, \
                 tc.tile_pool(name="gat2", bufs=6) as gat2p, \
                 tc.tile_pool(name="ohp", bufs=3) as ohp, \
                 tc.tile_pool(name="sm", bufs=6) as sm, \
                 tc.tile_pool(name="hp", bufs=3) as hp, \
                 tc.tile_pool(name="op", bufs=3) as op, \
                 tc.tile_pool(name="ps1", bufs=4, space="PSUM") as ps1, \
                 tc.tile_pool(name="ps2", bufs=2, space="PSUM") as ps2:

                # ---- layer 1
                for g in range(G):
                    gat = gatp.tile([128, C * D1], f32, tag="gat")
                    sec_gather(gat, gidx_t[:], g, u1_full, D1, 2 * g)

                    db = sm.tile([128, W], f32, tag="db")
                    nc.sync.dma_start(out=db[:], in_=disb_in[g])

                    oh = ohp.tile([128, C * W], f32, tag="oh")
                    nc.vector.tensor_tensor(
                        out=oh[:].rearrange("p (c w) -> p c w", w=W),
                        in0=dstl_t[:, g * C:(g + 1) * C][:, :, None
                            ].to_broadcast([128, C, W]),
                        in1=iota_t[:][:, None, :].to_broadcast([128, C, W]),
                        op=mybir.AluOpType.is_equal,
                    )
                    aggT = ps1.tile([D1, W], f32, space="PSUM", tag="aggT")
                    for c in range(C):
                        nc.tensor.matmul(
                            out=aggT[:],
                            lhsT=gat[:, c * D1:(c + 1) * D1],
                            rhs=oh[:, c * W:(c + 1) * W],
                            start=(c == 0), stop=(c == C - 1),
                        )
                    h1 = hp.tile([D1, W], f32, tag="h1")
                    nc.vector.tensor_tensor(
                        out=h1[:], in0=aggT[:], in1=db[:],
                        op=mybir.AluOpType.mult)
                    h1b = hp.tile([D1, W], f32, tag="h1b")
                    nc.scalar.activation(h1b[:], h1[:], Relu,
                                         bias=b1c_t[:, 0:1], scale=1.0)
                    u2ps = ps2.tile([W, D2], f32, space="PSUM", tag="u2ps")
                    nc.tensor.matmul(out=u2ps[:], lhsT=h1b[:], rhs=w2_t[:],
                                     start=True, stop=True)
                    u2t = op.tile([W, D2], f32, tag="u2t")
                    nc.scalar.activation(u2t[:], u2ps[:], Copy,
                                         scale=discols_t[:, g:g + 1])
                    nc.sync.dma_start(out=u2_stage[g * W:(g + 1) * W, :],
                                      in_=u2t[:])

                nc.gpsimd.collective_compute(
                    "AllGather", mybir.AluOpType.bypass, replica_groups=RG,
                    ins=[u2_stage[:]], outs=[u2_full[:]],
                )

                # ---- layer 2
                for g in range(G):
                    gat2 = gat2p.tile([128, C * D2], f32, tag="gat2")
                    sec_gather(gat2, gidx_t[:], g, u2_full, D2, 2 * g)

                    oh = ohp.tile([128, C * W], f32, tag="oh")
                    nc.vector.tensor_tensor(
                        out=oh[:].rearrange("p (c w) -> p c w", w=W),
                        in0=dstl_t[:, g * C:(g + 1) * C][:, :, None
                            ].to_broadcast([128, C, W]),
                        in1=iota_t[:][:, None, :].to_broadcast([128, C, W]),
                        op=mybir.AluOpType.is_equal,
                    )
                    agg2 = ps2.tile([W, D2], f32, space="PSUM", tag="agg2")
                    for c in range(C):
                        nc.tensor.matmul(
                            out=agg2[:],
                            lhsT=oh[:, c * W:(c + 1) * W],
                            rhs=gat2[:, c * D2:(c + 1) * D2],
                            start=(c == 0), stop=(c == C - 1),
                        )
                    o1 = op.tile([W, D2], f32, tag="o1")
                    nc.scalar.activation(o1[:], agg2[:], Copy,
                                         scale=discols_t[:, g:g + 1])
                    o2 = op.tile([W, D2], f32, tag="o2")
                    nc.vector.tensor_tensor(
                        out=o2[:], in0=o1[:], in1=b2r_t[:],
                        op=mybir.AluOpType.add)
                    nc.sync.dma_start(out=out_ext[g * W:(g + 1) * W, :],
                                      in_=o2[:])

    nc.compile()
    return nc


def kernel(x_node_types, edge_index, emb, W1, b1, W2, b2):
    from concourse.bass_utils import run_bass_kernel_spmd

    pre = preprocess(x_node_types, edge_index, emb, W1, b1, W2, b2)
    nc = build_program(pre["CA"], pre["CB"])

    in_maps = []
    for c in range(NCORE):
        in_maps.append({
            "embw1": pre["embW1"], "w2": pre["w2"],
            "gidx": pre["gidx"][c], "dstlpc": pre["dstl_pc"][c],
            "utidx": pre["ut_idx"][c], "dispc": pre["dis_pc"][c],
            "disb": pre["disb"][c], "discols": pre["dis_cols"][c],
            "iota": pre["iota_bc"], "b1c": pre["b1c"], "b2r": pre["b2r"],
        })

    trace = bool(int(os.environ.get("BASS_KERNEL_TRACE", "0")))
    res = run_bass_kernel_spmd(nc, in_maps, list(range(NCORE)), trace=trace)
    if trace and res.exec_time_ns is not None:
        print(f"HW exec time: {res.exec_time_ns} ns")

    out = np.zeros((N_NODES, D2), np.float32)
    s2n = pre["slot2node"]
    for c in range(NCORE):
        valid = s2n[c] >= 0
        out[s2n[c, valid]] = res.results[c]["out"][valid]
    return out



# revision 9
# speedup vs baseline: 1.2826x; 1.2826x over previous
"""GCN encoder (2-layer GCNConv + embedding lookup) on 8 trn2 NeuronCores.

Strategy (graph-parallel, per sharding hint):
  - Nodes are sharded across the 8 cores by id (12500 each), then renumbered
    into "slots": each core packs its nodes into G groups of <=128 nodes with
    balanced edge counts (bin-packing on host), giving a uniform SPMD program.
  - norm folding: out = dis .* segsum(u[src]) + b with u = dis .* (x @ W),
    dis = deg^-1/2 (self-loops included as ordinary edges).
  - u1 = dis * embW1[type] (embW1 = emb@W1 folded on host) computed per shard,
    AllGather -> full u1 table; per-group edge gathers (dma_gather, int16 idx
    with +-32768 midpoint base trick, 2 tables) feed one-hot matmul
    aggregation accumulated in PSUM; layer-2 the same with u2 = dis*(h1@W2).
"""
import os
import sys

sys.path.insert(0, "/opt/trn_rl_repo")
import ml_dtypes
import numpy as np

BF16 = ml_dtypes.bfloat16

N_NODES = 100000
NCORE = 8
NPC = N_NODES // NCORE          # 12500 nodes per core
D1, D2 = 128, 64
NTYPES = 1000
G = 104                         # groups per core
W = 128                         # slots (nodes) per group
SLOTS = G * W                   # 13312 slots per core
TOTAL_SLOTS = NCORE * SLOTS     # 106496
BASE_A, BASE_B = 32768, 73728   # gather base rows (midpoint trick)
PAD_DST = 512.0                 # dst_local sentinel (exact in bf16) -> zero one-hot row
GB = 4                          # groups per dma_gather call

# Chunked u_full layout: 8 chunks of groups; chunk k of all cores is one
# contiguous row block (so per-chunk AllGathers write contiguous slices and
# overlap with compute).  Chunks 0-4 cover groups 0..63 = "A half" -> rows
# [0, 65536) reachable from BASE_A; groups 64..103 = "B half" -> rows
# [65536, 106496) reachable from BASE_B.
CHB = [0, 13, 26, 39, 52, 64, 77, 90, 104]      # chunk boundaries (groups)
NCHUNK = len(CHB) - 1
RB = [NCORE * 128 * b for b in CHB]             # row base per chunk
GA = 64                         # groups in the A half (per core)
NA_NODES = 7692                 # nodes per core assigned to the A half


# ---------------------------------------------------------------- host prep
def _pack_core(nodes, degA, degB):
    """Greedy 2-d balanced packing of `nodes` into G groups of <=W nodes.
    Returns list of per-group node arrays."""
    a = degA[nodes].astype(np.float64)
    b = degB[nodes].astype(np.float64)
    order = np.argsort(-(a + b), kind="stable")
    tA = max(a.sum() / G, 1.0)
    tB = max(b.sum() / G, 1.0)
    sA = np.zeros(G)
    sB = np.zeros(G)
    cnt = np.zeros(G, np.int64)
    assign = np.empty(len(nodes), np.int64)
    for i in order:
        cost = np.maximum((sA + a[i]) / tA, (sB + b[i]) / tB)
        cost[cnt >= W] = np.inf
        g = int(np.argmin(cost))
        assign[i] = g
        sA[g] += a[i]
        sB[g] += b[i]
        cnt[g] += 1
    # refinement: push per-group section maxima down toward ceil(target/128)
    # by moving nodes out of the worst group into groups with slack.
    capA = max(1024, int(np.ceil(sA.max() / 128)) * 128 - 128)
    capB = max(1024, int(np.ceil(sB.max() / 128)) * 128 - 128)
    for _ in range(4000):
        worstA = sA.max() > capA
        s = sA if worstA else sB
        cap = capA if worstA else capB
        gsrc = int(np.argmax(s))
        if s[gsrc] <= cap:
            break
        members = np.where(assign == gsrc)[0]
        need = s[gsrc] - cap
        vals = (a if worstA else b)[members]
        cand = members[np.argsort(-(vals - need) * (vals >= need) - vals)]
        moved = False
        for i in cand[:20]:
            ai, bi = a[i], b[i]
            ok = (cnt < W) & (sA + ai <= capA) & (sB + bi <= capB)
            ok[gsrc] = False
            if ok.any():
                load = np.where(ok, np.maximum((sA + ai) / capA,
                                               (sB + bi) / capB), np.inf)
                gdst = int(np.argmin(load))
                assign[i] = gdst
                sA[gsrc] -= ai; sB[gsrc] -= bi; cnt[gsrc] -= 1
                sA[gdst] += ai; sB[gdst] += bi; cnt[gdst] += 1
                moved = True
                break
        if not moved:
            break
    groups = [nodes[assign == g] for g in range(G)]
    return groups


def preprocess(x_node_types, edge_index, emb, W1, b1, W2, b2):
    types = np.asarray(x_node_types).astype(np.int64)
    src = np.asarray(edge_index[0]).astype(np.int64)
    dst = np.asarray(edge_index[1]).astype(np.int64)
    loop = np.arange(N_NODES, dtype=np.int64)
    src_all = np.concatenate([src, loop])
    dst_all = np.concatenate([dst, loop])

    deg = np.bincount(dst_all, minlength=N_NODES).astype(np.float32)
    dis = (1.0 / np.sqrt(deg)).astype(np.float32)   # deg >= 1 (self loops)

    a_mask = src_all < SPLIT_NODE
    degA = np.bincount(dst_all[a_mask], minlength=N_NODES)
    degB = np.bincount(dst_all[~a_mask], minlength=N_NODES)

    # pack nodes -> slots
    slot_of = np.full(N_NODES, -1, np.int64)
    slot2node = np.full((NCORE, SLOTS), -1, np.int64)
    for c in range(NCORE):
        nodes = np.arange(c * NPC, (c + 1) * NPC, dtype=np.int64)
        groups = _pack_core(nodes, degA, degB)
        for g, gn in enumerate(groups):
            s0 = g * W
            slot_of[gn] = c * SLOTS + s0 + np.arange(len(gn))
            slot2node[c, s0:s0 + len(gn)] = gn

    # per-edge data
    e_srcslot = slot_of[src_all]
    e_dstslot = slot_of[dst_all]
    e_core = e_dstslot // SLOTS
    e_grp = (e_dstslot % SLOTS) // W
    e_dloc = e_dstslot % W
    e_sec = (src_all >= SPLIT_NODE).astype(np.int64)   # 0 = A, 1 = B

    bucket = (e_core * G + e_grp) * 2 + e_sec
    nb = NCORE * G * 2
    counts = np.bincount(bucket, minlength=nb)
    cA = counts.reshape(NCORE, G, 2)[:, :, 0]
    cB = counts.reshape(NCORE, G, 2)[:, :, 1]
    CA = int(np.ceil(cA.max() / 128))
    CB = int(np.ceil(cB.max() / 128))
    C = CA + CB

    # stable order by bucket; rank within bucket
    order = np.argsort(bucket, kind="stable")
    offs = np.zeros(nb + 1, np.int64)
    np.cumsum(counts, out=offs[1:])
    rank = np.arange(len(bucket)) - offs[bucket[order]]

    # padded per-(core,group,sec) slot position -> chunk & partition
    nsec = np.array([CA * 128, CB * 128])
    secbase = np.array([0, CA * 128])
    q = rank + secbase[e_sec[order]]          # slot index within group space
    part = q % 128
    chunk = q // 128

    # fill idx + dstl arrays (padded defaults)
    idx_val = np.zeros((NCORE, G, C * 128), np.int16)     # 0 = safe row at base
    dstl = np.full((NCORE, G, 128, C), PAD_DST, np.float32)
    oc = e_core[order]
    og = e_grp[order]
    osec = e_sec[order]
    oslot = e_srcslot[order]
    base = np.where(osec == 0, BASE_A, BASE_B)
    iv = oslot - base
    assert iv.min() >= -32768 and iv.max() <= 32767
    idx_val[oc, og, q] = iv.astype(np.int16)
    dstl[oc, og, part, chunk] = e_dloc[order].astype(np.float32)
    # guarantee the LAST index of each section is >= 0 (gen truncates
    # trailing negatives): pad slots already 0; if a real edge with negative
    # idx lands exactly at the section end, swap it with a pad... instead just
    # check and fix by appending nothing: force last element of each section
    # to be a pad when the section is full AND negative.
    lastA = idx_val[:, :, CA * 128 - 1]
    lastB = idx_val[:, :, C * 128 - 1]
    if (lastA < 0).any() or (lastB < 0).any():
        # swap the offending last element with the first non-negative element
        for c, g in zip(*np.where(lastA < 0)):
            sec = idx_val[c, g, :CA * 128]
            j = int(np.argmax(sec >= 0))
            _swap_edge(idx_val, dstl, c, g, CA * 128 - 1, j)
        for c, g in zip(*np.where(lastB < 0)):
            sec = idx_val[c, g, CA * 128:]
            j = CA * 128 + int(np.argmax(sec >= 0))
            _swap_edge(idx_val, dstl, c, g, C * 128 - 1, j)

    # wrapped int16 layout [128, n/16] (16-partition blocks replicated 8x)
    def wrap(vals):  # vals [..., n] -> [..., 128, n//16]
        n = vals.shape[-1]
        w = vals.reshape(*vals.shape[:-1], n // 16, 16)
        w = np.swapaxes(w, -1, -2)              # [..., 16, n//16]
        return np.tile(w, (1,) * (vals.ndim - 1) + (8, 1))

    # batched gather inputs: GB groups per dma_gather call, A and B sections
    # concatenated separately (wrap is 16-blockwise, so wrap(concat)=concat(wrap))
    idxA = idx_val[:, :, :CA * 128].reshape(NCORE, G // GB, GB * CA * 128)
    idxB = idx_val[:, :, CA * 128:].reshape(NCORE, G // GB, GB * CB * 128)
    gidxA = np.ascontiguousarray(wrap(idxA))   # [NCORE, G/GB, 128, GB*CA*8]
    gidxB = np.ascontiguousarray(wrap(idxB))   # [NCORE, G/GB, 128, GB*CB*8]
    gidx = np.ascontiguousarray(wrap(idx_val).transpose(0, 2, 1, 3).reshape(
        NCORE, 128, G * C * 8))                # [NCORE, 128, G*C*8]
    dstl_pc = np.ascontiguousarray(
        dstl.transpose(0, 2, 1, 3).reshape(NCORE, 128, G * C))

    # u1-phase gather: idx-list position i writes tile [p=i%128, c=i//128],
    # and the store DMA maps tile [p, c] -> u1_stage row p*NU+c.
    NU = SLOTS // 128
    ii = np.arange(SLOTS)
    slot_at_i = (ii % 128) * NU + ii // 128
    types_slot = np.zeros((NCORE, SLOTS), np.int64)
    dis_slot = np.zeros((NCORE, SLOTS), np.float32)
    for c in range(NCORE):
        valid = slot2node[c] >= 0
        types_slot[c, valid] = types[slot2node[c, valid]]
        dis_slot[c, valid] = dis[slot2node[c, valid]]
    ut_idx = wrap(types_slot[:, slot_at_i].astype(np.int16))  # [NCORE, 128, SLOTS//16]
    # dis_pc[c][p][j] = dis of slot p*NU+j
    dis_pc = dis_slot[:, (np.arange(128)[:, None] * NU + np.arange(NU)[None, :])]

    disb = np.ascontiguousarray(
        np.broadcast_to(
            dis_slot.reshape(NCORE, G, 1, W), (NCORE, G, 128, W)
        )
    ).astype(np.float32)
    dis_cols = np.ascontiguousarray(
        dis_slot.reshape(NCORE, G, W).transpose(0, 2, 1)
    ).astype(np.float32)                                   # [NCORE, 128, G]

    embW1 = (np.asarray(emb, np.float32) @ np.asarray(W1, np.float32)).astype(BF16)
    iota_bc = np.tile(np.arange(W, dtype=np.float32)[None, :], (128, 1)).astype(BF16)
    b1c = np.asarray(b1, np.float32).reshape(128, 1)
    b2r = np.tile(np.asarray(b2, np.float32)[None, :], (128, 1))

    return dict(
        CA=CA, CB=CB, dis=dis, slot2node=slot2node,
        gidxA=gidxA, gidxB=gidxB, gidx=gidx,
        dstl_pc=dstl_pc.astype(BF16),
        dstl=np.ascontiguousarray(dstl.reshape(NCORE, G, 128, C)),
        ut_idx=np.ascontiguousarray(ut_idx),
        dis_pc=np.ascontiguousarray(dis_pc.astype(BF16)),
        disb=disb, dis_cols=dis_cols,
        embW1=embW1, iota_bc=iota_bc, b1c=b1c, b2r=b2r,
        w2=np.asarray(W2, np.float32).astype(BF16),
    )


def _swap_edge(idx_val, dstl, c, g, i, j):
    C = dstl.shape[-1]
    idx_val[c, g, i], idx_val[c, g, j] = idx_val[c, g, j], idx_val[c, g, i]
    pi, ci, pj, cj = i % 128, i // 128, j % 128, j // 128
    t = dstl[c, g, pi, ci]
    dstl[c, g, pi, ci] = dstl[c, g, pj, cj]
    dstl[c, g, pj, cj] = t


# ---------------------------------------------------------------- device
def build_program(CA, CB):
    from concourse import bacc, mybir, tile

    C = CA + CB
    f32, i16 = mybir.dt.float32, mybir.dt.int16
    bf16 = mybir.dt.bfloat16
    NU = SLOTS // 128
    NBATCH = G // GB

    nc = bacc.Bacc(None, target_bir_lowering=False, num_devices=NCORE,
                   num_swdge_queues=4)
    embw1_in = nc.dram_tensor("embw1", [NTYPES, D1], bf16, kind="ExternalInput")
    w2_in = nc.dram_tensor("w2", [D1, D2], bf16, kind="ExternalInput")
    gidx_in = nc.dram_tensor("gidx", [128, G * (CA + CB) * 8], i16,
                             kind="ExternalInput")
    dstlpc_in = nc.dram_tensor("dstlpc", [128, G * (CA + CB)], bf16,
                               kind="ExternalInput")
    utidx_in = nc.dram_tensor("utidx", [128, SLOTS // 16], i16, kind="ExternalInput")
    dispc_in = nc.dram_tensor("dispc", [128, NU], bf16, kind="ExternalInput")
    disb_in = nc.dram_tensor("disb", [G, 128, W], f32, kind="ExternalInput")
    discols_in = nc.dram_tensor("discols", [128, G], f32, kind="ExternalInput")
    iota_in = nc.dram_tensor("iota", [128, W], bf16, kind="ExternalInput")
    b1c_in = nc.dram_tensor("b1c", [128, 1], f32, kind="ExternalInput")
    b2r_in = nc.dram_tensor("b2r", [128, D2], f32, kind="ExternalInput")
    out_ext = nc.dram_tensor("out", [SLOTS, D2], f32, kind="ExternalOutput")

    u1_stage = nc.dram_tensor("u1_stage", [SLOTS, D1], bf16)
    u1_full = nc.dram_tensor("u1_full", [TOTAL_SLOTS, D1], bf16, addr_space="Shared")
    u2_stage = nc.dram_tensor("u2_stage", [SLOTS, D1], bf16)
    u2_full = nc.dram_tensor("u2_full", [TOTAL_SLOTS, D1], bf16, addr_space="Shared")

    RG = [list(range(NCORE))]
    Relu = mybir.ActivationFunctionType.Relu
    Copy = mybir.ActivationFunctionType.Copy

    def sec_gather(gat, gq, g, table, elem, base_q):
        """Two per-group section gathers; idx slices of the resident gidx tile."""
        o = g * (CA + CB) * 8
        nc.gpsimd.dma_gather(
            out_ap=gat[:, :CA * elem].rearrange("p (c d) -> p c d", d=elem),
            in_ap=table[BASE_A:, :],
            idxs_ap=gq[:, o:o + CA * 8],
            num_idxs=CA * 128, num_idxs_reg=CA * 128,
            elem_size=elem, single_packet=False, queue_num=base_q % 4,
        )
        nc.gpsimd.dma_gather(
            out_ap=gat[:, CA * elem:].rearrange("p (c d) -> p c d", d=elem),
            in_ap=table[BASE_B:, :],
            idxs_ap=gq[:, o + CA * 8:o + (CA + CB) * 8],
            num_idxs=CB * 128, num_idxs_reg=CB * 128,
            elem_size=elem, single_packet=False, queue_num=(base_q + 1) % 4,
        )

    C_ = C

    ccw_in = nc.dram_tensor("ccw_in", [1, 128], f32)
    ccw_out = nc.dram_tensor("ccw_out", [NCORE, 128], f32, addr_space="Shared")

    with tile.TileContext(nc) as tc:
        with tc.tile_pool(name="cst", bufs=1) as cst:
            nc.gpsimd.collective_compute(
                "AllGather", mybir.AluOpType.bypass, replica_groups=RG,
                ins=[ccw_in[:]], outs=[ccw_out[:]],
            )
            w2_t = cst.tile([D1, D2], bf16)
            nc.sync.dma_start(out=w2_t[:], in_=w2_in[:])
            iota_t = cst.tile([128, W], bf16)
            nc.sync.dma_start(out=iota_t[:], in_=iota_in[:])
            b1c_t = cst.tile([128, 1], f32)
            nc.sync.dma_start(out=b1c_t[:], in_=b1c_in[:])
            b2r_t = cst.tile([128, D2], f32)
            nc.sync.dma_start(out=b2r_t[:], in_=b2r_in[:])
            discols_t = cst.tile([128, G], f32)
            nc.sync.dma_start(out=discols_t[:], in_=discols_in[:])
            gidx_t = cst.tile([128, G * C * 8], i16)
            nc.sync.dma_start(out=gidx_t[:], in_=gidx_in[:])
            dstl_t = cst.tile([128, G * C], bf16)
            nc.sync.dma_start(out=dstl_t[:], in_=dstlpc_in[:])

            # ---- u1 = dis * embW1[type] (own pool, freed before main loops)
            with tc.tile_pool(name="u1p", bufs=1) as u1p:
                dispc_t = u1p.tile([128, NU], bf16)
                nc.sync.dma_start(out=dispc_t[:], in_=dispc_in[:])
                utidx_t = u1p.tile([128, SLOTS // 16], i16)
                nc.sync.dma_start(out=utidx_t[:], in_=utidx_in[:])
                u1t = u1p.tile([128, NU * D1], bf16)
                NQ4 = NU // 4          # chunks per queue-split gather
                for j in range(4):
                    nc.gpsimd.dma_gather(
                        out_ap=u1t[:, j * NQ4 * D1:(j + 1) * NQ4 * D1
                                   ].rearrange("p (c d) -> p c d", d=D1),
                        in_ap=embw1_in[:],
                        idxs_ap=utidx_t[:, j * (SLOTS // 64):(j + 1) * (SLOTS // 64)],
                        num_idxs=SLOTS // 4, num_idxs_reg=SLOTS // 4,
                        elem_size=D1, single_packet=False, queue_num=j,
                    )
                nc.vector.tensor_tensor(
                    out=u1t[:].rearrange("p (c d) -> p c d", d=D1),
                    in0=u1t[:].rearrange("p (c d) -> p c d", d=D1),
                    in1=dispc_t[:][:, :, None].to_broadcast([128, NU, D1]),
                    op=mybir.AluOpType.mult,
                )
                nc.sync.dma_start(
                    out=u1_stage[:].rearrange("(p c) d -> p c d", p=128),
                    in_=u1t[:].rearrange("p (c d) -> p c d", d=D1),
                )
            nc.gpsimd.collective_compute(
                "AllGather", mybir.AluOpType.bypass, replica_groups=RG,
                ins=[u1_stage[:]], outs=[u1_full[:]],
            )

            with tc.tile_pool(name="gat", bufs=8) as gatp, \
                 tc.tile_pool(name="gat2", bufs=6) as gat2p, \
                 tc.tile_pool(name="ohp", bufs=3) as ohp, \
                 tc.tile_pool(name="sm", bufs=6) as sm, \
                 tc.tile_pool(name="hp", bufs=3) as hp, \
                 tc.tile_pool(name="op", bufs=3) as op, \
                 tc.tile_pool(name="ps1", bufs=4, space="PSUM") as ps1, \
                 tc.tile_pool(name="ps2", bufs=2, space="PSUM") as ps2:

                # ---- layer 1
                for g in range(G):
                    gat = gatp.tile([128, C * D1], bf16, tag="gat")
                    sec_gather(gat, gidx_t[:], g, u1_full, D1, 2 * g)

                    db = sm.tile([128, W], f32, tag="db")
                    nc.sync.dma_start(out=db[:], in_=disb_in[g])

                    oh = ohp.tile([128, C * W], bf16, tag="oh")
                    nc.vector.tensor_tensor(
                        out=oh[:].rearrange("p (c w) -> p c w", w=W),
                        in0=dstl_t[:, g * C:(g + 1) * C][:, :, None
                            ].to_broadcast([128, C, W]),
                        in1=iota_t[:][:, None, :].to_broadcast([128, C, W]),
                        op=mybir.AluOpType.is_equal,
                    )
                    aggT = ps1.tile([D1, W], f32, space="PSUM", tag="aggT")
                    for c in range(C):
                        nc.tensor.matmul(
                            out=aggT[:],
                            lhsT=gat[:, c * D1:(c + 1) * D1],
                            rhs=oh[:, c * W:(c + 1) * W],
                            start=(c == 0), stop=(c == C - 1),
                        )
                    h1 = hp.tile([D1, W], f32, tag="h1")
                    nc.vector.tensor_tensor(
                        out=h1[:], in0=aggT[:], in1=db[:],
                        op=mybir.AluOpType.mult)
                    h1b = hp.tile([D1, W], bf16, tag="h1b")
                    nc.scalar.activation(h1b[:], h1[:], Relu,
                                         bias=b1c_t[:, 0:1], scale=1.0)
                    u2ps = ps2.tile([W, D2], f32, space="PSUM", tag="u2ps")
                    nc.tensor.matmul(out=u2ps[:], lhsT=h1b[:], rhs=w2_t[:],
                                     start=True, stop=True)
                    # u2 rows are padded to D1 bf16 cols so layer-2 gather
                    # rows stay >=256B; cols D2..D1 are never matmul'd.
                    u2t = op.tile([W, D1], bf16, tag="u2t")
                    nc.scalar.activation(u2t[:, 0:D2], u2ps[:], Copy,
                                         scale=discols_t[:, g:g + 1])
                    nc.sync.dma_start(out=u2_stage[g * W:(g + 1) * W, :],
                                      in_=u2t[:])

                nc.gpsimd.collective_compute(
                    "AllGather", mybir.AluOpType.bypass, replica_groups=RG,
                    ins=[u2_stage[:]], outs=[u2_full[:]],
                )

                # ---- layer 2
                for g in range(G):
                    gat2 = gat2p.tile([128, C * D1], bf16, tag="gat2")
                    sec_gather(gat2, gidx_t[:], g, u2_full, D1, 2 * g)

                    oh = ohp.tile([128, C * W], bf16, tag="oh")
                    nc.vector.tensor_tensor(
                        out=oh[:].rearrange("p (c w) -> p c w", w=W),
                        in0=dstl_t[:, g * C:(g + 1) * C][:, :, None
                            ].to_broadcast([128, C, W]),
                        in1=iota_t[:][:, None, :].to_broadcast([128, C, W]),
                        op=mybir.AluOpType.is_equal,
                    )
                    agg2 = ps2.tile([W, D2], f32, space="PSUM", tag="agg2")
                    for c in range(C):
                        nc.tensor.matmul(
                            out=agg2[:],
                            lhsT=oh[:, c * W:(c + 1) * W],
                            rhs=gat2[:, c * D1:c * D1 + D2],
                            start=(c == 0), stop=(c == C - 1),
                        )
                    o1 = op.tile([W, D2], f32, tag="o1")
                    nc.scalar.activation(o1[:], agg2[:], Copy,
                                         scale=discols_t[:, g:g + 1])
                    o2 = op.tile([W, D2], f32, tag="o2")
                    nc.vector.tensor_tensor(
                        out=o2[:], in0=o1[:], in1=b2r_t[:],
                        op=mybir.AluOpType.add)
                    nc.sync.dma_start(out=out_ext[g * W:(g + 1) * W, :],
                                      in_=o2[:])

    nc.compile()
    return nc


def kernel(x_node_types, edge_index, emb, W1, b1, W2, b2):
    from concourse.bass_utils import run_bass_kernel_spmd

    pre = preprocess(x_node_types, edge_index, emb, W1, b1, W2, b2)
    nc = build_program(pre["CA"], pre["CB"])

    in_maps = []
    for c in range(NCORE):
        in_maps.append({
            "embw1": pre["embW1"], "w2": pre["w2"],
            "gidx": pre["gidx"][c], "dstlpc": pre["dstl_pc"][c],
            "utidx": pre["ut_idx"][c], "dispc": pre["dis_pc"][c],
            "disb": pre["disb"][c], "discols": pre["dis_cols"][c],
            "iota": pre["iota_bc"], "b1c": pre["b1c"], "b2r": pre["b2r"],
        })

    trace = bool(int(os.environ.get("BASS_KERNEL_TRACE", "0")))
    res = run_bass_kernel_spmd(nc, in_maps, list(range(NCORE)), trace=trace)
    if trace and res.exec_time_ns is not None:
        print(f"HW exec time: {res.exec_time_ns} ns")

    out = np.zeros((N_NODES, D2), np.float32)
    s2n = pre["slot2node"]
    for c in range(NCORE):
        valid = s2n[c] >= 0
        out[s2n[c, valid]] = res.results[c]["out"][valid]
    return out

